# revision 3
# baseline (speedup 1.0000x reference)
"""Trainium2 Bass kernel for nn_LinearTransmissionLayer.

Computes out[b, n, :] = mean(X[b], axis=0) @ B + cT  (broadcast over n).

Full inputs: X [16, 4096, 1024] f32, B [1024, 1024] f32, cT [1, 1024] f32.
Sharding: data-parallel over batch dim, 2 batches per core on 8 cores.
B and cT replicated. Per-core device program does the full computation
including the broadcast write of the output (memory-bound: 32 MiB in +
32 MiB out + 4 MiB B per core).
"""

import os
import numpy as np

_CACHE = {}

# Problem constants (hardcoded; kernel.py must be self-contained)
BT, N, FIN, FOUT = 16, 4096, 1024, 1024
NCORES = 8
BPC = BT // NCORES  # batches per core
P = 128  # SBUF partitions
RREAD = 512  # X rows per read DMA  (512*1024*4 = 2 MiB)
WRITE = 512  # out rows per write DMA (2 MiB)


def _build():
    import concourse.bass as bass
    import concourse.tile as tile
    from concourse import bacc, mybir

    dt = mybir.dt.float32
    nc = bacc.Bacc("TRN2", target_bir_lowering=False, debug=False)

    X = nc.dram_tensor("X", [BPC, N, FIN], dt, kind="ExternalInput")
    Bm = nc.dram_tensor("B", [FIN, FOUT], dt, kind="ExternalInput")
    cT = nc.dram_tensor("cT", [1, FOUT], dt, kind="ExternalInput")
    OUT = nc.dram_tensor("OUT", [BPC, N, FOUT], dt, kind="ExternalOutput")

    G = RREAD // P  # row-groups per read tile
    NRT = N // RREAD  # read tiles per batch
    NJ = FIN // P  # 8 chunks of the contraction dim

    with tile.TileContext(nc) as tc:
        with (
            tc.tile_pool(name="const", bufs=1) as cpool,
            tc.tile_pool(name="xin", bufs=4) as xpool,
            tc.tile_pool(name="small", bufs=2) as spool,
            tc.tile_pool(name="bc", bufs=2) as bcpool,
            tc.tile_pool(name="ps", bufs=1, space=bass.MemorySpace.PSUM) as pspool,
        ):
            ones_col = cpool.tile([P, 1], dt, tag="ones_col")
            nc.vector.memset(ones_col[:], 1.0)
            ones_row = cpool.tile([1, P], dt, tag="ones_row")
            nc.vector.memset(ones_row[:], 1.0)
            ct_sb = cpool.tile([1, FOUT], dt, tag="ct")
            nc.sync.dma_start(out=ct_sb[:], in_=cT[:])
            # B laid out as [128, 8*1024]: B_sb[p, j*1024 + f] = B[j*128 + p, f]
            b_sb = cpool.tile([P, NJ * FOUT], dt, tag="bmat")
            nc.sync.dma_start(
                out=b_sb[:], in_=Bm[:].rearrange("(j p) f -> p j f", p=P)
            )

            for b in range(BPC):
                # ---- column sums of X[b] (sum over all N rows) ----
                ps_cs = pspool.tile([1, FIN], dt, tag="ps_cs")
                for k in range(NRT):
                    xt = xpool.tile([P, G * FIN], dt, tag="xt")
                    nc.sync.dma_start(
                        out=xt[:],
                        in_=X[b, k * RREAD : (k + 1) * RREAD, :].rearrange(
                            "(g p) f -> p g f", p=P
                        ),
                    )
                    for g in range(G):
                        for h in range(2):
                            nc.tensor.matmul(
                                ps_cs[0:1, h * 512 : (h + 1) * 512],
                                lhsT=ones_col[:, 0:1],
                                rhs=xt[:, g * FIN + h * 512 : g * FIN + (h + 1) * 512],
                                start=(k == 0 and g == 0),
                                stop=(k == NRT - 1 and g == G - 1),
                            )
                # ---- mean row (scale by 1/N), still [1, FIN] free-layout ----
                mean_row = spool.tile([1, FIN], dt, tag="mean_row")
                for h in range(2):
                    nc.vector.tensor_scalar_mul(
                        mean_row[:, h * 512 : (h + 1) * 512],
                        ps_cs[0:1, h * 512 : (h + 1) * 512],
                        1.0 / N,
                    )
                # ---- transpose mean to [128, NJ] (Fin down partitions) ----
                ps_t = pspool.tile([P, NJ], dt, tag="ps_t")
                for j in range(NJ):
                    nc.tensor.matmul(
                        ps_t[:, j : j + 1],
                        lhsT=mean_row[0:1, j * P : (j + 1) * P],
                        rhs=ones_col[0:1, 0:1],
                        start=True,
                        stop=True,
                    )
                mean_col = spool.tile([P, NJ], dt, tag="mean_col")
                nc.vector.tensor_copy(mean_col[:], ps_t[:])
                # ---- row = mean @ B  ([1, FOUT]) ----
                ps_r = pspool.tile([1, FOUT], dt, tag="ps_r")
                for j in range(NJ):
                    for h in range(2):
                        nc.tensor.matmul(
                            ps_r[0:1, h * 512 : (h + 1) * 512],
                            lhsT=mean_col[:, j : j + 1],
                            rhs=b_sb[:, j * FOUT + h * 512 : j * FOUT + (h + 1) * 512],
                            start=(j == 0),
                            stop=(j == NJ - 1),
                        )
                out_row = spool.tile([1, FOUT], dt, tag="out_row")
                for h in range(2):
                    nc.vector.tensor_add(
                        out_row[:, h * 512 : (h + 1) * 512],
                        ps_r[0:1, h * 512 : (h + 1) * 512],
                        ct_sb[:, h * 512 : (h + 1) * 512],
                    )
                # ---- broadcast row across 128 partitions (outer product) ----
                ps_b = pspool.tile([P, FOUT], dt, tag="ps_b")
                for h in range(2):
                    nc.tensor.matmul(
                        ps_b[:, h * 512 : (h + 1) * 512],
                        lhsT=ones_row[0:1, :],
                        rhs=out_row[0:1, h * 512 : (h + 1) * 512],
                        start=True,
                        stop=True,
                    )
                bcast = bcpool.tile([P, FOUT], dt, tag="bcast")
                nc.vector.tensor_copy(bcast[:], ps_b[:])
                # ---- broadcast-write the batch's output ----
                for k in range(N // WRITE):
                    nc.sync.dma_start(
                        out=OUT[b, k * WRITE : (k + 1) * WRITE, :].rearrange(
                            "(g p) f -> p g f", p=P
                        ),
                        in_=bcast[:, None, :].to_broadcast([P, WRITE // P, FOUT]),
                    )

    nc.compile()
    return nc


def _get_nc():
    if "nc" not in _CACHE:
        _CACHE["nc"] = _build()
    return _CACHE["nc"]


def kernel(X, B, cT):
    from concourse import bass_utils

    nc = _get_nc()
    X = np.ascontiguousarray(X, dtype=np.float32)
    B = np.ascontiguousarray(B, dtype=np.float32)
    cT = np.ascontiguousarray(cT, dtype=np.float32)

    in_maps = [
        {"X": X[c * BPC : (c + 1) * BPC], "B": B, "cT": cT} for c in range(NCORES)
    ]
    trace = bool(int(os.environ.get("KERNEL_TRACE", "0")))
    res = bass_utils.run_bass_kernel_spmd(
        nc,
        in_maps,
        core_ids=list(range(NCORES)),
        trace=trace,
    )
    _CACHE["last_result"] = res
    out = np.concatenate([r["OUT"] for r in res.results], axis=0)
    return out
